# revision 47
# baseline (speedup 1.0000x reference)
"""Trainium2 Bass kernel for nn_MLPModel_70703751626902 (moe_routing).

Per-robot hypernetwork MLP: each of 1024 samples routes to one of 32
per-robot weight sets (input hypernet 624->256, three 256x256 hidden
layers, output hypernet 256->24).

Strategy (expert-parallel): group samples by robot on the host, shard
robots across the 8 cores (4 robots/core, one per "slot"), so every
core runs dense per-robot matmuls with only its own robots' weights
(~5.8MB/core instead of 46MB replicated). Activations stay transposed
([hidden, batch]) the whole way so each layer's PSUM output feeds the
next layer's moving operand directly. The obs mask is folded into the
transposed input with one elementwise multiply; the masked input bias
(maskbar @ bi) rides inside the input-layer matmul itself (maskbar
rows appended to the input, bi rows appended to Wi); all other biases
ride along as per-partition bias operands of the PSUM->SBUF
relu/copy activation ops.

All DRAM tensors are packed host-side so every DMA moves >=2KB
contiguous runs per partition (128-partition-major packing of the
contraction dim).

Samples for slot j occupy columns [off_j, off_j + cap_j) where cap_j is
the max sample count over the 8 robots assigned to slot j (rounded up
to 8); robots are assigned to slots by descending count so padding
waste is small. All 8 cores run an identical program (SPMD).
"""

import numpy as np

F32 = np.float32

# matmul operand dtype: "f32" (exact, ~60us), "f32r" (fp32 bits, PE
# tf32-like fast path, rel err ~2e-4, ~44us), "bf16" (half DMA bytes,
# 1cyc/row PE, rel err ~3e-3, ~30us)
W_DT = "f32r"


def _plan(ids, n_robots):
    """Group samples by robot and assign robots to (core, slot)."""
    counts = np.bincount(ids, minlength=n_robots)
    order = np.argsort(-counts, kind="stable")
    n_slots = (n_robots + 7) // 8
    caps = []
    for j in range(n_slots):
        grp = order[8 * j : 8 * j + 8]
        m = int(counts[grp].max()) if len(grp) else 0
        caps.append(max(8, int(np.ceil(max(m, 1) / 8) * 8)))
    offs = np.concatenate([[0], np.cumsum(caps)]).astype(int)
    nb = int(offs[-1])
    assert nb <= 512, f"batch columns per core {nb} exceeds PSUM bank"
    rows = [[None] * n_slots for _ in range(8)]
    robot_at = [[None] * n_slots for _ in range(8)]
    for rank, robot in enumerate(order):
        j, c = rank // 8, rank % 8
        if j >= n_slots:
            break
        rows[c][j] = np.nonzero(ids == robot)[0]
        robot_at[c][j] = int(robot)
    return {
        "caps": tuple(caps),
        "offs": tuple(int(o) for o in offs),
        "nb": nb,
        "rows": rows,
        "robot_at": robot_at,
        "n_slots": n_slots,
    }


def _pack_kp(a, ncols=None):
    """[K, M] -> [128, ceil(K/128)*M]; col kt*M+m holds a[kt*128+p, m]."""
    k, m = a.shape
    nk = (k + 127) // 128
    out = np.zeros((128, nk * m), a.dtype)
    for kt in range(nk):
        ks = min(128, k - kt * 128)
        out[:ks, kt * m : kt * m + m] = a[kt * 128 : kt * 128 + ks, :]
    return out


_PROGRAM_CACHE = {}


def _build_program(caps, kin, seq, hid, kout, w_dt_name):
    import concourse.mybir as mybir
    import concourse.tile as tile
    from concourse import bacc

    f32 = mybir.dt.float32
    wdt = {"f32": f32, "f32r": mybir.dt.float32r, "bf16": mybir.dt.bfloat16}[w_dt_name]
    n_slots = len(caps)
    offs = np.concatenate([[0], np.cumsum(caps)]).astype(int)
    nb = int(offs[-1])
    # input-layer contraction: obs rows (kin) plus seq maskbar rows that
    # carry the masked input bias (bi rows ride in wi) — see host prep
    kaug = kin + seq
    nk = (kin + 127) // 128
    assert kaug <= nk * 128, "maskbar fold needs slack in the last chunk"
    klast = kaug - 128 * (nk - 1)
    nh = hid // 128  # hidden column halves

    import concourse.bass as bass_mod

    # Skip the framework's init-time all-engine barrier: it only
    # protects the const-AP memsets, which this kernel never reads
    # (every activation bias is a real SBUF column). Without it the
    # DMA queues start issuing ~4us earlier instead of waiting for the
    # slowest engine's program load. All data hazards are still covered
    # by Tile-generated semaphores, and the kernel-exit drain/barriers
    # are emitted after the patch is restored.
    _orig_barrier = bass_mod.Bass.all_engine_barrier
    bass_mod.Bass.all_engine_barrier = lambda self, *, sem_only=False: None
    try:
        nc = bacc.Bacc("TRN2", target_bir_lowering=False, debug=False, num_devices=8)
    finally:
        bass_mod.Bass.all_engine_barrier = _orig_barrier

    # xt and mexp interleaved in two pieces [xtA|meA|xtB|meB] so the
    # first input-layer chunks can start before the whole input lands
    nka = min(3, nk)
    nkb = nk - nka
    xtme_d = nc.dram_tensor("xtme", [128, 2 * nk * nb], wdt, kind="ExternalInput")
    bc_d = nc.dram_tensor("bcols", [128, n_slots * 8], f32, kind="ExternalInput")
    # weights packed slot-major in single tensors; DMAs pull column
    # ranges (groups of slots) so descriptor size and arrival order can
    # be tuned: slot 0 alone first (compute starts sooner), the rest in
    # bigger chunks (fatter descriptors, fewer serial dma_start issues)
    wiw = nk * hid  # wi columns per slot
    whw = 3 * nh * hid  # wh columns per slot
    # wi pieces in units of hid-column chunks (kt-granular): slot 0 is
    # split A/B so the first matmuls start before its tail arrives
    if n_slots > 1:
        wi_pieces = [(0, nka), (nka, nk)]
        if w_dt_name == "bf16":
            wi_pieces += [(nk, n_slots * nk)]
            wh_groups = [(0, 2), (2, n_slots)] if n_slots > 2 else [(0, n_slots)]
        else:
            wi_pieces += [(j * nk, (j + 1) * nk) for j in range(1, n_slots)]
            wh_groups = [(j, j + 1) for j in range(n_slots)]
    else:
        wi_pieces = [(0, nka), (nka, nk)] if nkb else [(0, nk)]
        wh_groups = [(0, 1)]
    wi_d = nc.dram_tensor("wi", [128, n_slots * wiw], wdt, kind="ExternalInput")
    wh_d = nc.dram_tensor("wh", [128, n_slots * whw], wdt, kind="ExternalInput")
    wo_d = nc.dram_tensor(
        "wo", [128, n_slots * nh * kout], wdt, kind="ExternalInput"
    )
    ot_d = nc.dram_tensor("ot", [kout, nb], f32, kind="ExternalOutput")

    relu = mybir.ActivationFunctionType.Relu
    ident = mybir.ActivationFunctionType.Identity
    act_parity = [0]

    with tile.TileContext(nc) as tc:
        with (
            tc.tile_pool(name="sb", bufs=1) as pool,
            tc.tile_pool(name="ps", bufs=4, space="PSUM") as psum,
            tc.tile_pool(name="pso", bufs=1, space="PSUM") as psum_o,
        ):
            # DMA issue is ~0.7us of sequencer time per instruction and
            # transfers drain FIFO per queue, so each queue gets its
            # DMAs in need-order; the two HWDGE queues issue in parallel.
            # sync engine: weight pieces in usage order
            wi_chunk, wh_slot = {}, {}
            for c0, c1 in wi_pieces:
                t = pool.tile([128, (c1 - c0) * hid], wdt, tag=f"wig{c0}")
                nc.sync.dma_start(t[:], wi_d[:, c0 * hid : c1 * hid])
                for c in range(c0, c1):
                    wi_chunk[c] = (t, (c - c0) * hid)
            for g0, g1 in wh_groups:
                t = pool.tile([128, (g1 - g0) * whw], wdt, tag=f"whg{g0}")
                nc.sync.dma_start(t[:], wh_d[:, g0 * whw : g1 * whw])
                for j in range(g0, g1):
                    wh_slot[j] = (t, (j - g0) * whw)

            def wi_lhsT(j, kt, h, ks):
                t, base = wi_chunk[j * nk + kt]
                o = base + h * 128
                return t[:ks, o : o + 128]

            def wh_lhsT(j, li, pi, h):
                t, base = wh_slot[j]
                o = base + li * nh * hid + pi * hid + h * 128
                return t[:, o : o + 128]

            # scalar engine: inputs (two pieces) + small tensors + wo
            xtme_a = pool.tile([128, 2 * nka * nb], wdt, tag="xtmea")
            nc.scalar.dma_start(xtme_a[:], xtme_d[:, : 2 * nka * nb])
            if nkb:
                xtme_b = pool.tile([128, 2 * nkb * nb], wdt, tag="xtmeb")
                nc.scalar.dma_start(xtme_b[:], xtme_d[:, 2 * nka * nb :])
            bc_t = pool.tile([128, n_slots * 8], f32, tag="bc")
            nc.scalar.dma_start(bc_t[:], bc_d[:, :])
            wo_t = pool.tile([128, n_slots * nh * kout], wdt, tag="wo")
            nc.scalar.dma_start(wo_t[:], wo_d[:, :])

            # masked transposed input (one packed multiply per piece)
            xm_a = pool.tile([128, nka * nb], wdt, tag="xma")
            nc.vector.tensor_mul(
                xm_a[:], xtme_a[:, : nka * nb], xtme_a[:, nka * nb :]
            )
            if nkb:
                xm_b = pool.tile([128, nkb * nb], wdt, tag="xmb")
                nc.vector.tensor_mul(
                    xm_b[:], xtme_b[:, : nkb * nb], xtme_b[:, nkb * nb :]
                )

            def xm_rhs(kt, ks, c0, w):
                if kt < nka:
                    return xm_a[:ks, kt * nb + c0 : kt * nb + c0 + w]
                return xm_b[:ks, (kt - nka) * nb + c0 : (kt - nka) * nb + c0 + w]

            def act_op(dst, src, func, bias):
                """PSUM->SBUF activation, alternating scalar/vector engines."""
                if act_parity[0] % 2 == 0:
                    nc.scalar.activation(dst, src, func, bias=bias)
                elif func is relu:
                    nc.vector.tensor_scalar(
                        dst, src, bias, 0.0,
                        mybir.AluOpType.add, mybir.AluOpType.max,
                    )
                else:
                    nc.vector.tensor_scalar(
                        dst, src, bias, None, mybir.AluOpType.add,
                    )
                act_parity[0] += 1

            # Layer 0: act0[h, b] = relu([xm; maskbar] @ [Wi; bi])
            act0 = pool.tile([128, nh * nb], wdt, tag="act0")
            p0 = [psum.tile([128, nb], f32, tag="ps", name=f"p0h{h}") for h in range(nh)]
            for j in range(n_slots):
                sl = slice(int(offs[j]), int(offs[j]) + caps[j])
                for h in range(nh):
                    for kt in range(nk):
                        ks = 128 if kt < nk - 1 else klast
                        nc.tensor.matmul(
                            p0[h][:, sl],
                            wi_lhsT(j, kt, h, ks),
                            xm_rhs(kt, ks, int(offs[j]), caps[j]),
                            start=(kt == 0), stop=(kt == nk - 1),
                        )
            zero_bias = bc_t[:, 7:8]  # unused bcols column, always zero
            for h in range(nh):
                act_op(act0[:, h * nb : (h + 1) * nb], p0[h][:, :], relu, zero_bias)

            # Hidden layers
            prev = act0
            for li in range(3):
                nxt = pool.tile([128, nh * nb], wdt, tag=f"act{li + 1}")
                for h in range(nh):
                    p = psum.tile([128, nb], f32, tag="ps")
                    for j in range(n_slots):
                        sl = slice(int(offs[j]), int(offs[j]) + caps[j])
                        for pi in range(nh):
                            nc.tensor.matmul(
                                p[:, sl],
                                wh_lhsT(j, li, pi, h),
                                prev[:, pi * nb + int(offs[j]) : pi * nb + int(offs[j]) + caps[j]],
                                start=(pi == 0), stop=(pi == nh - 1),
                            )
                    for j in range(n_slots):
                        sl = slice(int(offs[j]), int(offs[j]) + caps[j])
                        bias = bc_t[:, j * 8 + li * 2 + h : j * 8 + li * 2 + h + 1]
                        act_op(
                            nxt[:, h * nb + int(offs[j]) : h * nb + int(offs[j]) + caps[j]],
                            p[:, sl], relu, bias,
                        )
                prev = nxt

            # Output layer (identity + bias)
            po = psum_o.tile([kout, nb], f32, tag="po")
            for j in range(n_slots):
                sl = slice(int(offs[j]), int(offs[j]) + caps[j])
                for pi in range(nh):
                    w0 = (j * nh + pi) * kout
                    nc.tensor.matmul(
                        po[:, sl],
                        wo_t[:, w0 : w0 + kout],
                        prev[:, pi * nb + int(offs[j]) : pi * nb + int(offs[j]) + caps[j]],
                        start=(pi == 0), stop=(pi == nh - 1),
                    )
            # two out tiles so the first half's DMA can start while the
            # second half's bias-adds still run
            jh = (n_slots + 1) // 2
            mid = int(offs[jh])
            ot_a = pool.tile([kout, mid], f32, tag="ota")
            ot_b = pool.tile([kout, nb - mid], f32, tag="otb")
            for j in range(n_slots):
                sl = slice(int(offs[j]), int(offs[j]) + caps[j])
                bias = bc_t[:kout, j * 8 + 6 : j * 8 + 7]
                if j < jh:
                    dst = ot_a[:, int(offs[j]) : int(offs[j]) + caps[j]]
                else:
                    dst = ot_b[:, int(offs[j]) - mid : int(offs[j]) - mid + caps[j]]
                act_op(dst, po[:, sl], ident, bias)
                if j == jh - 1:
                    nc.sync.dma_start(ot_d[:, :mid], ot_a[:])
            nc.sync.dma_start(ot_d[:, mid:], ot_b[:])

    nc.compile()
    return nc


def _get_program(caps, kin, seq, hid, kout, w_dt_name):
    key = (caps, kin, seq, hid, kout, w_dt_name)
    if key not in _PROGRAM_CACHE:
        _PROGRAM_CACHE[key] = _build_program(caps, kin, seq, hid, kout, w_dt_name)
    return _PROGRAM_CACHE[key]


def _np_wdt(w_dt_name):
    if w_dt_name == "bf16":
        import ml_dtypes

        return np.dtype(ml_dtypes.bfloat16)
    return np.dtype(np.float32)


def _prep_core_inputs(plan, c, obs, maskbar, Wi, bi, W1, b1, W2, b2, W3, b3, Wo, bo,
                      w_dt_name):
    seq = maskbar.shape[1]
    kin = obs.shape[1]
    lobs = kin // seq
    hid = Wi.shape[3]
    kout = seq * Wo.shape[3]
    n_slots = plan["n_slots"]
    nb = plan["nb"]
    offs = plan["offs"]
    nk = (kin + 127) // 128
    nh = hid // 128
    wnp = _np_wdt(w_dt_name)

    kaug = kin + seq  # obs rows + maskbar rows (carry the input bias)
    xt = np.zeros((kaug, nb), F32)
    mexp = np.zeros((kaug, nb), F32)
    mexp[kin:, :] = 1.0
    bc = np.zeros((128, n_slots * 8), F32)
    wi = np.zeros((128, n_slots * nk * hid), F32)
    wh = np.zeros((128, n_slots * 3 * nh * hid), F32)
    wo = np.zeros((128, n_slots * nh * kout), F32)

    for j in range(n_slots):
        r = plan["robot_at"][c][j]
        if r is None:
            continue
        rows = plan["rows"][c][j]
        n = len(rows)
        o0 = offs[j]
        if n:
            xt[:kin, o0 : o0 + n] = obs[rows].T
            mb = maskbar[rows]
            mexp[:kin, o0 : o0 + n] = np.repeat(mb, lobs, axis=1).T
            xt[kin:, o0 : o0 + n] = mb.T
        o2 = j * nk * hid
        wi[:, o2 : o2 + nk * hid] = _pack_kp(
            np.vstack([Wi[r].reshape(kin, hid), bi[r]])
        )
        o2 = j * 3 * nh * hid
        for li, W in enumerate((W1, W2, W3)):
            wh[:, o2 + li * nh * hid : o2 + (li + 1) * nh * hid] = _pack_kp(W[r])
        wo[:, j * nh * kout : (j + 1) * nh * kout] = _pack_kp(
            Wo[r].transpose(1, 0, 2).reshape(hid, kout)
        )
        for li, bvec in enumerate((b1[r], b2[r], b3[r])):
            for h in range(nh):
                bc[:, j * 8 + li * 2 + h] = bvec[h * 128 : (h + 1) * 128]
        bc[:kout, j * 8 + 6] = bo[r].reshape(-1)

    nka = min(3, nk)
    xtp, mep = _pack_kp(xt), _pack_kp(mexp)
    xtme = np.concatenate(
        [xtp[:, : nka * nb], mep[:, : nka * nb],
         xtp[:, nka * nb :], mep[:, nka * nb :]], axis=1,
    )
    return {
        "xtme": xtme.astype(wnp),
        "bcols": bc,
        "wi": wi.astype(wnp),
        "wh": wh.astype(wnp),
        "wo": wo.astype(wnp),
    }


def _unshard(plan, results, B, kout):
    out = np.zeros((B, kout), F32)
    offs = plan["offs"]
    for c in range(8):
        ot = results[c]["ot"]
        for j in range(plan["n_slots"]):
            rows = plan["rows"][c][j]
            if rows is None or len(rows) == 0:
                continue
            o0 = offs[j]
            out[rows] = np.asarray(ot[:, o0 : o0 + len(rows)], F32).T
    return out


def kernel(obs, obs_mask, unimal_ids, Wi, bi, W1, b1, W2, b2, W3, b3, Wo, bo,
           _runner=None, _w_dt=None):
    w_dt_name = _w_dt or W_DT
    obs = np.asarray(obs, F32)
    obs_mask = np.asarray(obs_mask)
    ids = np.asarray(unimal_ids).astype(np.int64)
    Wi, bi = np.asarray(Wi, F32), np.asarray(bi, F32)
    W1, b1 = np.asarray(W1, F32), np.asarray(b1, F32)
    W2, b2 = np.asarray(W2, F32), np.asarray(b2, F32)
    W3, b3 = np.asarray(W3, F32), np.asarray(b3, F32)
    Wo, bo = np.asarray(Wo, F32), np.asarray(bo, F32)

    B = obs.shape[0]
    n_robots = Wi.shape[0]
    seq, lobs, hid = Wi.shape[1], Wi.shape[2], Wi.shape[3]
    kin = seq * lobs
    kout = seq * Wo.shape[3]
    maskbar = 1.0 - obs_mask.astype(F32)

    plan = _plan(ids, n_robots)
    nc = _get_program(plan["caps"], kin, seq, hid, kout, w_dt_name)

    in_maps = [
        _prep_core_inputs(plan, c, obs, maskbar, Wi, bi, W1, b1, W2, b2, W3, b3,
                          Wo, bo, w_dt_name)
        for c in range(8)
    ]

    if _runner is None:
        from concourse.bass_utils import run_bass_kernel_spmd

        res = run_bass_kernel_spmd(nc, in_maps, core_ids=list(range(8)))
        results = res.results
    else:
        results = _runner(nc, in_maps)

    return _unshard(plan, results, B, kout)
